# revision 22
# baseline (speedup 1.0000x reference)
"""Trainium2 Bass kernel for nn_MixClassificationBigSNN_Alt.

Network (per reference): ConstantCurrentLIF encoder (T=32) -> 3 LIF layers
(2048->512->512->256) -> LI readout (256->100); output = readout membrane
voltage at t=32.

Strategy (v2 — wire-optimized):
- Data-parallel over batch: 2048 rows -> 8 cores x 256.
- The axon tunnel to the device runs at ~60 MB/s, so host->device bytes
  dominate wall time. v1 shipped 110 MB per call (weights replicated 8x as
  f32 hi/lo pairs + f32 activations); v2 ships ~10 MB:
  * The encoder is evaluated EXACTLY on the host: the constant-current LIF
    spike train is periodic with period kstar = first threshold-crossing
    step; kstar is recovered by a 32-level threshold staircase whose
    thresholds are bisected against the exact fp32 recurrence. khat
    (= 32 - kstar + 1 clamped) fits in uint8 -> 0.5 MB/core instead of a
    2 MB f32 activation tensor. The device rebuilds the 32-bit spike
    pattern word with integer shift-doubling as in v1.
  * Weights travel as fp16 hi/lo pairs (hi = fp16(w); lo = fp16((w-hi)*2^11),
    pre-scaled so it stays in fp16 normal range) and are converted on-device
    to the same f32r hi/lo representation v1 used (fp16 values are exact in
    f32r's 11-bit significand, so the proven matmul path is unchanged).
  * The packed weight blob [128 x 22928] is SHARDED across the 8 cores on
    the wire (16 rows each) and AllGathered HBM->HBM on-device over
    NeuronLink -> 0.73 MB/core instead of 5.9 MB replicated.
- The jitted PJRT executable is cached across calls (v1 re-traced and
  re-compiled the XLA wrapper on every invocation).
- All matmuls run on the PE in float32r with hi+lo accumulating passes
  (~21 effective mantissa bits). Synaptic currents i live in PSUM in
  natural units; membrane potentials v live in SBUF; spikes are computed
  as Relu(Sign(v - vth)) on the Scalar engine.
"""
import numpy as np
import sys

for _p in ("/opt/trn_rl_repo", "/root/.axon_site/_ro/trn_rl_repo"):
    if _p not in sys.path:
        sys.path.insert(0, _p)

import contextlib
import concourse.bass as bass
import concourse.bacc as bacc
import concourse.tile as tile
from concourse import mybir

f32 = mybir.dt.float32
f32r = mybir.dt.float32r
f16 = mybir.dt.float16
i32 = mybir.dt.int32
u8 = mybir.dt.uint8
u16 = mybir.dt.uint16
AT = mybir.AluOpType
AF = mybir.ActivationFunctionType

T = 32
VTH = np.float32(0.33)
NCORES = 8
B = 2048
BPC = B // NCORES            # 256 batch rows per core
FIN = 2048
H1, H2, H3, NOUT = 512, 512, 256, 100
NFC = FIN // 128             # 16 input-feature chunks
F = NFC * BPC                # 4096 free elements in the [128, F] layout

# state tensor free-dim layout: [V1 (4*256) | V2 (4*256) | V3 (2*256) | VO (256)]
OFF1, OFF2, OFF3, OFFO = 0, 1024, 2048, 2560
WIDTH = 2816                 # total free width of V/I state tensors
ZW = 2560                    # spiking portion (V1|V2|V3)

# packed fp16 weight blob layout (free-dim offsets; each weight is
# [hi-half | lo-half] of equal width)
W1W = 2 * NFC * 4 * 128      # 16384
W2W = 2 * 4 * 4 * 128        # 4096
W3W = 2 * 4 * 2 * 128        # 2048
WOW = 2 * 2 * NOUT           # 400
OW1, OW2, OW3, OWO = 0, W1W, W1W + W2W, W1W + W2W + W3W
WTOT = W1W + W2W + W3W + WOW  # 22928
RSH = 128 // NCORES          # 16 blob rows shipped per core
LO_SCALE = float(2.0 ** -11)

# khat wire pack: 3 base-33 digits per uint16 word (planar thirds of the
# [128, F] layout). Unpacked on-device with exhaustively-verified
# magic-number divisions: v//1089 == (v*30813)>>25, v//33 == (v*1986)>>16.
KW = (F + 2) // 3            # 1366 words per partition
KP2 = F - 2 * KW             # 1364 = width of the third plane

_runner_cache = {}


def _crossing_step(c):
    v = np.float32(0.0)
    for k in range(1, T + 1):
        v = np.float32(v + np.float32(np.float32(0.1) * np.float32(c - v)))
        if v > VTH:
            return k
    return 1000


def _bisect_thresholds():
    """theta_k (fp32, decreasing): c > theta_k  <=>  encoder spikes within <= k steps,
    exactly matching the fp32 recurrence v += 0.1*(c-v)."""
    thetas = []
    for k in range(1, T + 1):
        lo, hi = np.float32(0.3), np.float32(4.0)
        assert _crossing_step(lo) > k and _crossing_step(hi) <= k
        while np.nextafter(lo, hi, dtype=np.float32) != hi:
            mid = np.float32((np.float64(lo) + np.float64(hi)) / 2)
            if mid == lo or mid == hi:
                mid = np.nextafter(lo, hi, dtype=np.float32)
            if _crossing_step(mid) <= k:
                hi = mid
            else:
                lo = mid
        thetas.append(lo)
    th = np.array(thetas, np.float32)
    assert np.all(np.diff(th) < 0)
    return th


def _pack_lhsT_f16(wT, kchunks, mchunks, mtile):
    """wT [K, M] fp32 -> fp16 hi/lo packed as [128, 2*kchunks*mchunks*mtile]
    with chunk (p, kc, mc) at free offset ((p*kchunks + kc)*mchunks + mc)*mtile.
    lo is pre-scaled by 2^11 to stay in fp16 normal range; the device
    multiplies it back by 2^-11 while converting to f32r."""
    K, M = wT.shape
    assert K == kchunks * 128 and M == mchunks * mtile
    hi = wT.astype(np.float16)
    lo = ((wT - hi.astype(np.float32)) * np.float32(2048.0)).astype(np.float16)
    halves = []
    for h in (hi, lo):
        halves.append(
            h.reshape(kchunks, 128, mchunks, mtile)
            .transpose(1, 0, 2, 3)
            .reshape(128, kchunks * mchunks * mtile)
        )
    return np.concatenate(halves, axis=1)


def _build_program():
    """Build + compile the SPMD bass program (no scalars baked in)."""
    nc = bacc.Bacc("TRN2", target_bir_lowering=False, debug=False,
                   num_devices=NCORES)

    kh_in = nc.dram_tensor("kh_in", [128, KW], u16, kind="ExternalInput").ap()
    wsh_in = nc.dram_tensor("wsh_in", [RSH, WTOT], f16, kind="ExternalInput").ap()
    vo_out = nc.dram_tensor("vo_out", [NOUT, BPC], f16, kind="ExternalOutput").ap()

    with tile.TileContext(nc) as tc:
        with contextlib.ExitStack() as ctx:
            # ---- weight shard gather: DRAM bounce -> AllGather -> full blob
            dram = ctx.enter_context(tc.tile_pool(name="dram", bufs=1, space="DRAM"))
            wsh_b = dram.tile([RSH, WTOT], f16, name="wsh_b")
            wg = dram.tile([128, WTOT], f16, name="wg")
            nc.gpsimd.dma_start(wsh_b[:], wsh_in)
            nc.gpsimd.collective_compute(
                "AllGather",
                AT.bypass,
                replica_groups=[list(range(NCORES))],
                ins=[wsh_b.opt()],
                outs=[wg.opt()],
            )

            # ---- persistent SBUF tiles
            wpool = ctx.enter_context(tc.tile_pool(name="wpool", bufs=1))
            w1 = wpool.tile([128, W1W], f32r, name="w1")
            w2 = wpool.tile([128, W2W], f32r, name="w2")
            w3 = wpool.tile([128, W3W], f32r, name="w3")
            wo = wpool.tile([128, WOW], f32r, name="wo")

            st = ctx.enter_context(tc.tile_pool(name="st", bufs=1))
            P = st.tile([128, F], i32, name="P")
            V = st.tile([128, WIDTH], f32, name="V")
            ip = ctx.enter_context(tc.tile_pool(name="ip", bufs=1, space="PSUM"))
            I = ip.tile([128, WIDTH], f32, name="I")
            bconst = st.tile([128, 1], f32, name="bconst")
            nc.vector.memset(bconst[:], -float(VTH))
            nc.vector.memset(V[:], 0.0)
            nc.vector.memset(I[:], 0.0)

            # ---- encoder pattern build from uint8 khat (overlaps the gather)
            with tc.tile_pool(name="enc", bufs=1) as enc:
                kh = enc.tile([128, KW], u16, name="kh", tag="slotE")
                nc.sync.dma_start(kh[:], kh_in)
                kv = enc.tile([128, KW], i32, name="kv", tag="slotF")
                nc.vector.tensor_copy(kv[:], kh[:])
                # unpack base-33 digits into the three planes of kint
                kint = enc.tile([128, F], i32, name="kint", tag="slotC")
                k2w = enc.tile([128, KW], i32, name="k2w", tag="slotG")
                nc.vector.tensor_scalar(k2w[:], kv[:], 30813, None, AT.mult)
                nc.vector.tensor_scalar(k2w[:], k2w[:], 25, None,
                                        AT.logical_shift_right)
                nc.vector.tensor_copy(kint[:, 2 * KW:F], k2w[:, 0:KP2])
                rem = enc.tile([128, KW], i32, name="rem", tag="slotH")
                nc.vector.scalar_tensor_tensor(rem[:], k2w[:], -1089, kv[:],
                                               AT.mult, AT.add)
                nc.vector.tensor_scalar(kint[:, KW:2 * KW], rem[:], 1986,
                                        None, AT.mult)
                nc.vector.tensor_scalar(kint[:, KW:2 * KW], kint[:, KW:2 * KW],
                                        16, None, AT.logical_shift_right)
                nc.vector.scalar_tensor_tensor(kint[:, 0:KW],
                                               kint[:, KW:2 * KW], -33, rem[:],
                                               AT.mult, AT.add)
                # ks = kstar = 33 - khat; P bit t-1 set iff kstar | t
                ks = enc.tile([128, F], i32, name="ks", tag="slotB")
                nc.vector.tensor_scalar(ks[:], kint[:], -1, 33, AT.mult, AT.add)
                ones_i = enc.tile([128, F], i32, name="ones_i", tag="slotA")
                nc.vector.memset(ones_i[:], 1)
                km = enc.tile([128, F], i32, name="km", tag="slotC")
                nc.vector.tensor_scalar(km[:], ks[:], 1, 31, AT.subtract, AT.min)
                u = enc.tile([128, F], i32, name="u", tag="slotD")
                nc.vector.tensor_tensor(u[:], ones_i[:], km[:], AT.logical_shift_left)
                sj = enc.tile([128, F], i32, name="sj", tag="slotC")
                vtmp = enc.tile([128, F], i32, name="vtmp", tag="slotA")
                for j in range(5):
                    nc.vector.tensor_scalar(sj[:], ks[:], 1 << j, 31, AT.mult, AT.min)
                    nc.vector.tensor_tensor(vtmp[:], u[:], sj[:], AT.logical_shift_left)
                    nc.vector.tensor_tensor(u[:], u[:], vtmp[:], AT.bitwise_or)
                m0 = enc.tile([128, F], i32, name="m0", tag="slotA")
                nc.vector.tensor_scalar(m0[:], ks[:], 32, None, AT.is_le)
                mneg = enc.tile([128, F], i32, name="mneg", tag="slotC")
                nc.vector.tensor_scalar(mneg[:], m0[:], -1, None, AT.mult)
                nc.vector.tensor_tensor(P[:], u[:], mneg[:], AT.bitwise_and)

            # ---- stage gathered fp16 blob into SBUF, convert to f32r hi/lo
            with tc.tile_pool(name="wstage", bufs=1) as wsg:
                wf16 = wsg.tile([128, WTOT], f16, name="wf16")
                nc.sync.dma_start(wf16[:], wg[:])
                for wt, off, width in ((w1, OW1, W1W), (w2, OW2, W2W),
                                       (w3, OW3, W3W), (wo, OWO, WOW)):
                    half = width // 2
                    nc.vector.tensor_copy(wt[:, 0:half], wf16[:, off:off + half])
                    nc.vector.tensor_scalar(wt[:, half:width],
                                            wf16[:, off + half:off + width],
                                            LO_SCALE, None, AT.mult)

            def mms(psum_slice, wtile, kchunks, mchunks, mtile, rhs_of_kc, oc):
                n = 0
                for p in range(2):
                    for kc in range(kchunks):
                        off = ((p * kchunks + kc) * mchunks + oc) * mtile
                        n += 1
                        nc.tensor.matmul(
                            psum_slice,
                            wtile[:, off:off + mtile],
                            rhs_of_kc(kc),
                            start=False,
                            stop=(n == 2 * kchunks),
                            skip_group_check=True,
                        )

            # ---- the scan
            with tc.tile_pool(name="work", bufs=2) as work:
                for t in range(1, T + 1):
                    # spike mask for this step from pattern words
                    zt_i = work.tile([128, F], i32, name="zt_i", tag="zt_i", bufs=1)
                    nc.vector.tensor_scalar(zt_i[:], P[:], t - 1, 1,
                                            AT.logical_shift_right, AT.bitwise_and)
                    zt = work.tile([128, F], f32r, name="zt", tag="zt")
                    nc.vector.tensor_copy(zt[:], zt_i[:])

                    # v_dec = 0.9*v + 0.1*i_old   (i_old: before this step's update)
                    nc.vector.tensor_scalar(V[:], V[:], 0.9, None, AT.mult)
                    nc.vector.scalar_tensor_tensor(V[:], I[:], 0.1, V[:],
                                                   AT.mult, AT.add)

                    # spikes z = Relu(Sign(v_dec - VTH)) for layers 1..3
                    sgn = work.tile([128, ZW], f32, name="sgn", tag="sgn", bufs=1)
                    nc.scalar.activation(sgn[:], V[:, 0:ZW], AF.Sign,
                                         bias=bconst[:], scale=1.0)
                    z123 = work.tile([128, ZW], f32r, name="z123", tag="z123")
                    nc.scalar.activation(z123[:], sgn[:], AF.Relu)

                    # reset: v = v_dec * (v_dec <= VTH)
                    nc.vector.scalar_tensor_tensor(V[:, 0:ZW], V[:, 0:ZW],
                                                   float(VTH), V[:, 0:ZW],
                                                   AT.is_le, AT.mult)

                    # i = 0.8*i + W z  (PSUM in place + PE accumulation)
                    nc.vector.tensor_scalar(I[:], I[:], 0.8, None, AT.mult)
                    for oc in range(4):
                        mms(I[:, OFF1 + oc * BPC: OFF1 + (oc + 1) * BPC], w1,
                            NFC, 4, 128, lambda kc: zt[:, kc * BPC:(kc + 1) * BPC], oc)
                    for oc in range(4):
                        mms(I[:, OFF2 + oc * BPC: OFF2 + (oc + 1) * BPC], w2,
                            4, 4, 128, lambda kc: z123[:, kc * BPC:(kc + 1) * BPC], oc)
                    for oc in range(2):
                        mms(I[:, OFF3 + oc * BPC: OFF3 + (oc + 1) * BPC], w3,
                            4, 2, 128,
                            lambda kc: z123[:, OFF2 + kc * BPC: OFF2 + (kc + 1) * BPC], oc)
                    mms(I[0:NOUT, OFFO:OFFO + BPC], wo,
                        2, 1, NOUT,
                        lambda kc: z123[:, OFF3 + kc * BPC: OFF3 + (kc + 1) * BPC], 0)

            # ---- output: vo at t=32 is V[0:100, OFFO:]
            oout = st.tile([NOUT, BPC], f16, name="oout")
            nc.vector.tensor_copy(oout[:], V[0:NOUT, OFFO:OFFO + BPC])
            nc.sync.dma_start(vo_out, oout[:])

    nc.compile()
    return nc


class _Runner:
    """Owns the compiled program and a persistently cached jitted PJRT
    executable (v1 re-traced + re-compiled the XLA wrapper every call)."""

    def __init__(self):
        import jax
        from jax.sharding import Mesh, PartitionSpec
        from jax.experimental.shard_map import shard_map
        from concourse.bass2jax import (
            install_neuronx_cc_hook, _bass_exec_p, partition_id_tensor)

        self.jax = jax
        nc = _build_program()
        self.nc = nc
        install_neuronx_cc_hook()

        partition_name = (nc.partition_id_tensor.name
                          if nc.partition_id_tensor else None)
        in_names, out_names, out_avals, zero_shapes = [], [], [], []
        for alloc in nc.m.functions[0].allocations:
            if not isinstance(alloc, mybir.MemoryLocationSet):
                continue
            name = alloc.memorylocations[0].name
            if alloc.kind == "ExternalInput":
                if name != partition_name:
                    in_names.append(name)
            elif alloc.kind == "ExternalOutput":
                shape = tuple(alloc.tensor_shape)
                dtype = mybir.dt.np(alloc.dtype)
                out_names.append(name)
                out_avals.append(jax.core.ShapedArray(shape, dtype))
                zero_shapes.append((shape, dtype))
        n_params = len(in_names)
        in_names_all = in_names + out_names + (
            [partition_name] if partition_name else [])
        donate = tuple(range(n_params, n_params + len(out_names)))

        def _body(*args):
            operands = list(args)
            if partition_name is not None:
                operands.append(partition_id_tensor())
            outs = _bass_exec_p.bind(
                *operands, out_avals=tuple(out_avals),
                in_names=tuple(in_names_all), out_names=tuple(out_names),
                lowering_input_output_aliases=(),
                sim_require_finite=True, sim_require_nnan=True, nc=nc)
            return tuple(outs)

        mesh = Mesh(np.asarray(jax.devices()[:NCORES]), ("core",))
        nio = n_params + len(out_names)
        self.sharded = jax.jit(
            shard_map(_body, mesh=mesh,
                      in_specs=(PartitionSpec("core"),) * nio,
                      out_specs=(PartitionSpec("core"),) * len(out_names),
                      check_rep=False),
            keep_unused=True)
        self.in_names = in_names
        self.out_names = out_names
        self.zero_shapes = zero_shapes

        # the kernel fully writes vo_out, so the operand buffers backing the
        # outputs are never read: create them on device ONCE and reuse
        # (no donation, no per-call host->device zero traffic)
        import jax.numpy as jnp
        shardspec = jax.sharding.NamedSharding(mesh, PartitionSpec("core"))
        self._zeros = jax.jit(
            lambda: tuple(jnp.zeros((NCORES * s[0], *s[1:]), d)
                          for s, d in zero_shapes),
            out_shardings=tuple(shardspec for _ in zero_shapes))()
        jax.block_until_ready(self._zeros)

    def run(self, concat_by_name):
        concat_in = [concat_by_name[nm] for nm in self.in_names]
        outs = self.sharded(*concat_in, *self._zeros)
        return {nm: np.asarray(outs[i]) for i, nm in enumerate(self.out_names)}


_theta_asc = None


def _prep_inputs(x, w1, w2, w3, w_out, fs, es):
    global _theta_asc
    if _theta_asc is None:
        _theta_asc = np.ascontiguousarray(_bisect_thresholds()[::-1])

    # ---- exact host-side encoder: khat = #{k: 2*fs*x > theta_k}
    two_fs = np.float32(np.float32(2.0) * fs)
    c = (x * two_fs).astype(np.float32, copy=False)
    khat = np.searchsorted(_theta_asc, c, side="left").astype(np.int32)
    # per-core [128, F] layout: [B, FIN] -> (core, b, kc, p) -> (core, p, kc, b)
    khc = np.ascontiguousarray(
        khat.reshape(NCORES, BPC, NFC, 128).transpose(0, 3, 2, 1)
    ).reshape(NCORES * 128, F)
    # base-33 pack: planes [0:KW), [KW:2KW), [2KW:F) -> one u16 word each
    k2 = np.zeros((NCORES * 128, KW), np.int32)
    k2[:, 0:KP2] = khc[:, 2 * KW:F]
    khc = (khc[:, 0:KW] + 33 * khc[:, KW:2 * KW] + 1089 * k2).astype(np.uint16)

    # ---- fp16 hi/lo packed weight blob, sharded over cores by blob row
    w1f = (np.float32(5.0) * es) * w1.T.astype(np.float32)   # [FIN, H1], folded 5*es
    blob = np.concatenate([
        _pack_lhsT_f16(np.ascontiguousarray(w1f), NFC, 4, 128),
        _pack_lhsT_f16(np.ascontiguousarray(w2.T.astype(np.float32)), 4, 4, 128),
        _pack_lhsT_f16(np.ascontiguousarray(w3.T.astype(np.float32)), 4, 2, 128),
        _pack_lhsT_f16(np.ascontiguousarray(w_out.T.astype(np.float32)), 2, 1, NOUT),
    ], axis=1)                                               # [128, WTOT] fp16
    # core r ships blob rows [16r, 16r+16); flat concat over cores == blob
    return {"kh_in": khc, "wsh_in": blob}


last_run_seconds = None


def kernel(x, w1, w2, w3, w_out, feature_scalar, encoder_scalar):
    global last_run_seconds
    import time
    x = np.asarray(x, np.float32)
    fs = np.float32(np.asarray(feature_scalar).reshape(-1)[0])
    es = np.float32(np.asarray(encoder_scalar).reshape(-1)[0])

    if "r" not in _runner_cache:
        _runner_cache["r"] = _Runner()
    runner = _runner_cache["r"]

    concat_by_name = _prep_inputs(
        x, np.asarray(w1, np.float32), np.asarray(w2, np.float32),
        np.asarray(w3, np.float32), np.asarray(w_out, np.float32), fs, es)

    t0 = time.perf_counter()
    res = runner.run(concat_by_name)
    last_run_seconds = time.perf_counter() - t0

    vo = res["vo_out"].astype(np.float32)                     # [8*NOUT, BPC]
    out = np.ascontiguousarray(
        vo.reshape(NCORES, NOUT, BPC).transpose(0, 2, 1)
    ).reshape(B, NOUT)
    return out
